# revision 5
# baseline (speedup 1.0000x reference)
"""Trainium2 Bass kernel v2 for nn_ActorNetSpiking.

Data-parallel over 8 cores (batch 4096 -> 512/core), feature-major layout.
Differences vs baseline:
- t-outer, layer-inner loop: single-step spike history (smaller SBUF), a
  layer-wavefront pipeline across engines.
- True-bias formulation: ScalarE drains PSUM->SBUF adding the per-row bias
  (bias + ns-rowsum) via per-partition bias AP. No beta table; threshold is
  the constant 0.5 and the reset needs no bias.
- Engine split per layer-step:
    PE   : banded bf16 matmuls (3-way hi/mid/lo split, fp32-exact)
    ACT  : syn = Identity(psum + c)            (PSUM drain + bias)
    DVE  : u = 0.5*u + syn                     (STT, merged per layer)
    DVE  : v = 0.75*v + u                      (STT in-place; v holds vt)
    DVE  : ns = (v <= 0.5) -> bf16 history     (TS 2x, merged per layer)
    Pool : v = v * ns                          (GPSIMD reset, merged)
- fc4 (2 units) computed batch-transposed: spikes as the stationary operand,
  so its LIF state is [128, 8] instead of [2, 512] (FD 8 instead of 512).
"""

import os
import numpy as np
import ml_dtypes

import concourse.bass as bass
import concourse.bacc as bacc_mod
import concourse.tile as tile
from concourse import mybir
from concourse._compat import with_exitstack
from concourse.bass_utils import run_bass_kernel_spmd

F32 = mybir.dt.float32
BF16 = mybir.dt.bfloat16
AF = mybir.ActivationFunctionType
OP = mybir.AluOpType

N_CORES = 8
B_FULL = 4096
B = B_FULL // N_CORES  # 512
T = 50
Tb = 4  # time block / scan-input DMA staging

CONV = [  # (Lin, Lout, Cin, Cout)
    (360, 178, 1, 5),
    (178, 87, 5, 5),
    (87, 42, 5, 5),
]
FC = [(216, 256), (256, 256), (256, 128)]  # fc4 handled separately


def _build_banded(w, b, Lin, Lout, Cin, Cout):
    rows_in, rows_out = Lin * Cin, Lout * Cout
    Wd = np.zeros((rows_in, rows_out), np.float32)
    K = w.shape[2]
    for l in range(Lout):
        for k in range(K):
            li = 2 * l + k
            Wd[li * Cin:(li + 1) * Cin, l * Cout:(l + 1) * Cout] = w[:, :, k].T
    bias = np.tile(b, Lout)
    return Wd, bias


def _plan_layers(inp):
    """Layers conv1..fc3 -> M-tiles with K-chunks, bias col (bias+rowsum)."""
    mats = []
    for i, (Lin, Lout, Cin, Cout) in enumerate(CONV):
        w, b = inp[f'conv{i+1}_w'], inp[f'conv{i+1}_b']
        mats.append(_build_banded(w, b, Lin, Lout, Cin, Cout))
    # fc1 with column permutation: row j (j<210) = (l3, co) -> ref idx co*42+l3
    fw, fb = inp['fc1_w'], inp['fc1_b']
    Wd = np.zeros((216, 256), np.float32)
    for j in range(210):
        l3, co = j // 5, j % 5
        Wd[j, :] = fw[:, co * 42 + l3]
    Wd[210:216, :] = fw[:, 210:216].T
    mats.append((Wd, fb.copy()))
    for i in (2, 3):
        fw, fb = inp[f'fc{i}_w'], inp[f'fc{i}_b']
        mats.append((fw.T.astype(np.float32), fb.copy()))

    layers = []
    for lidx, (Wd, bias) in enumerate(mats):
        rows_in, rows_out = Wd.shape
        ns_rows = np.zeros(rows_in, bool)
        if lidx >= 1:
            ns_rows[:] = True
            if lidx == 3:
                ns_rows[210:216] = False  # raw normal-spike rows
        rowsum = (Wd.astype(np.float64) * ns_rows[:, None]).sum(axis=0)
        Wd = Wd.copy()
        Wd[ns_rows, :] *= -1.0
        tiles = []
        for m0 in range(0, rows_out, 128):
            m1 = min(m0 + 128, rows_out)
            nz = np.nonzero(np.any(Wd[:, m0:m1] != 0.0, axis=1))[0]
            k0, k1 = int(nz.min()), int(nz.max()) + 1
            chunks = []
            for g in range(k0 // 128, (k1 + 127) // 128):
                a = g * 128
                bnd = min(a + 128, k1)
                chunks.append((a, bnd, Wd[a:bnd, m0:m1]))
            c = bias[m0:m1].astype(np.float64) + rowsum[m0:m1]
            tiles.append(dict(m0=m0, m1=m1, chunks=chunks,
                              bias=c.astype(np.float32)))
        layers.append(dict(rows_in=rows_in, rows_out=rows_out, tiles=tiles,
                           G_out=(rows_out + 127) // 128))
    return layers


def _pack_weights(layers):
    """All lhsT chunks -> one [128, 3*total] bf16 (hi/mid/lo); bias table."""
    total = 0
    for L in layers:
        for tl in L['tiles']:
            for (a, b_, Wc) in tl['chunks']:
                total += Wc.shape[1]
    wpack = np.zeros((128, total), np.float32)
    off = 0
    ti = 0
    for L in layers:
        for tl in L['tiles']:
            tl['offs'] = []
            tl['tidx'] = ti
            ti += 1
            for (a, b_, Wc) in tl['chunks']:
                K, M = Wc.shape
                wpack[:K, off:off + M] = Wc
                tl['offs'].append(off)
                off += M
    ntiles = ti
    btab = np.zeros((128, ntiles), np.float32)
    for L in layers:
        for tl in L['tiles']:
            btab[:tl['m1'] - tl['m0'], tl['tidx']] = tl['bias']
    hi = wpack.astype(ml_dtypes.bfloat16).astype(np.float32)
    mid = (wpack - hi).astype(ml_dtypes.bfloat16).astype(np.float32)
    lo = (wpack - hi - mid).astype(ml_dtypes.bfloat16)
    w3 = np.concatenate([hi.astype(ml_dtypes.bfloat16),
                         mid.astype(ml_dtypes.bfloat16), lo], axis=1)
    return w3, btab


def _pack_fc4(inp):
    """fc4 in batch-transposed form: rhs [128, 6] = -(fc4_w).T 3-split;
    bias row [1, 24] = tiled (fc4_b + rowsum) 3-split."""
    w4 = np.asarray(inp['fc4_w'], np.float64)   # (2, 128)
    b4 = np.asarray(inp['fc4_b'], np.float64)
    c4 = b4 + w4.sum(axis=1)                     # ns-encoding rowsum
    Wt = (-w4.T)                                 # [128, 2]
    hi = Wt.astype(ml_dtypes.bfloat16).astype(np.float64)
    mid = (Wt - hi).astype(ml_dtypes.bfloat16).astype(np.float64)
    lo = (Wt - hi - mid).astype(ml_dtypes.bfloat16)
    wf4 = np.concatenate([hi.astype(ml_dtypes.bfloat16),
                          mid.astype(ml_dtypes.bfloat16), lo],
                         axis=1)                 # [128, 6]
    c4t = np.tile(c4, 4)                         # [8] cols (j, a)
    chi = c4t.astype(ml_dtypes.bfloat16).astype(np.float64)
    cmid = (c4t - chi).astype(ml_dtypes.bfloat16).astype(np.float64)
    clo = (c4t - chi - cmid).astype(ml_dtypes.bfloat16)
    c4s = np.concatenate([chi.astype(ml_dtypes.bfloat16),
                          cmid.astype(ml_dtypes.bfloat16), clo])[None, :]  # [1,24]
    return wf4, c4s


# hist output slot offsets per layer (conv1..fc3): 7+4+2+2+2+1 = 18 groups
G_OUT = [7, 4, 2, 2, 2, 1]
HOFF = [0, 7, 11, 13, 15, 17]


@with_exitstack
def _emit(ctx, tc, layers, wcols, prm):
    nc = tc.nc
    persist = ctx.enter_context(tc.tile_pool(name="persist", bufs=1))
    scanp = ctx.enter_context(tc.tile_pool(name="scanin", bufs=2))
    psum = ctx.enter_context(tc.tile_pool(name="psum", bufs=3, space="PSUM"))
    psum4 = ctx.enter_context(tc.tile_pool(name="psum4", bufs=2, space="PSUM"))

    wsb = persist.tile([128, wcols], BF16, tag="wsb")
    nc.sync.dma_start(wsb[:], prm['w'][:])
    ntiles = sum(len(L['tiles']) for L in layers)
    bsb = persist.tile([128, ntiles], F32, tag="bsb")
    nc.sync.dma_start(bsb[:], prm['bias'][:])
    wf4 = persist.tile([128, 6], BF16, tag="wf4")
    nc.sync.dma_start(wf4[:], prm['wf4'][:])
    c4s = persist.tile([1, 24], BF16, tag="c4s")
    nc.sync.dma_start(c4s[:], prm['c4s'][:])
    ones = persist.tile([1, 128], BF16, tag="ones")
    nc.vector.memset(ones[:], 1.0)
    # materialize fc4 bias row broadcast to [128, 8] fp32 (one-time)
    c4sb = persist.tile([128, 8], F32, tag="c4sb")

    # spike history: 18 groups [128, B] bf16, layer-contiguous, double-
    # buffered across steps so the GPSIMD reset (reader of step t) never
    # blocks the next step's threshold write.
    hist = persist.tile([128, 18, Tb, B], BF16, tag="hist")
    # per-layer persistent states (u, v/vt) fp32 + transient syn fp32
    u_all = persist.tile([128, 18 * B], F32, tag="u")
    v_all = persist.tile([128, 18 * B], F32, tag="v")
    nc.gpsimd.memset(u_all[:], 0.0)
    nc.gpsimd.memset(v_all[:], 0.0)
    nc.gpsimd.memset(hist[:], 0.0)
    synp = ctx.enter_context(tc.tile_pool(name="synp", bufs=3))

    # fc4 transposed state [128, 8] (cols = (batch_slice j, unit a))
    u4 = persist.tile([128, 8], F32, tag="u4")
    v4 = persist.tile([128, 8], F32, tag="v4")
    ns4 = persist.tile([128, 8], BF16, tag="ns4")
    acc4 = persist.tile([128, 8], F32, tag="acc4")
    syn4 = persist.tile([128, 8], F32, tag="syn4")
    nc.vector.memset(u4[:], 0.0)
    nc.vector.memset(v4[:], 0.0)
    nc.vector.memset(acc4[:], 0.0)

    nblocks = (T + Tb - 1) // Tb
    for blk in range(nblocks):
        t0 = blk * Tb
        tbn = min(Tb, T - t0)
        sc = scanp.tile([128, 3, Tb, B], BF16, tag="scan")
        for g in range(3):
            p = min(128, 360 - g * 128)
            nc.sync.dma_start(sc[:p, g, :tbn, :],
                              prm['scan'][g * 128:g * 128 + p, t0:t0 + tbn, :])
        # normal spikes go straight into fc1's input rows (conv3-out slot 1,
        # partitions 82:88) -- ns(conv3) only writes [:M] so never clobbers
        nc.sync.dma_start(hist[82:88, HOFF[2] + 1, :tbn, :],
                          prm['normal'][:, t0:t0 + tbn, :])

        for li, L in enumerate(layers):
            g_out = L['G_out']
            tiles = L['tiles']
            ho = HOFF[li] * B
            u_l = u_all[:, ho:ho + g_out * B]
            v_l = v_all[:, ho:ho + g_out * B]
            for t in range(tbn):
                for ph in range(0, len(tiles), 2):
                    grp = tiles[ph:ph + 2]
                    gw = len(grp)
                    ps = psum.tile([128, 2 * B], F32, tag="ps2")
                    syn_t = synp.tile([128, 2 * B], F32, tag="synt")
                    for si, tl in enumerate(grp):
                        M = tl['m1'] - tl['m0']
                        nch = len(tl['chunks'])
                        for ci_, ((a, b_, Wc), off) in enumerate(
                                zip(tl['chunks'], tl['offs'])):
                            K = b_ - a
                            g_src, p_src = a // 128, a % 128
                            if li == 0:
                                rhs = sc[p_src:p_src + K, g_src, t, :]
                            else:
                                gi = HOFF[li - 1] + g_src
                                rhs = hist[p_src:p_src + K, gi, t, :]
                            for half in range(3):
                                nc.tensor.matmul(
                                    ps[:M, si * B:(si + 1) * B],
                                    wsb[:K, half * (wcols // 3) + off:
                                        half * (wcols // 3) + off + Wc.shape[1]],
                                    rhs,
                                    start=(ci_ == 0 and half == 0),
                                    stop=(ci_ == nch - 1 and half == 2))
                        nc.scalar.activation(
                            syn_t[:M, si * B:(si + 1) * B],
                            ps[:M, si * B:(si + 1) * B],
                            AF.Identity,
                            bias=bsb[:M, tl['tidx']:tl['tidx'] + 1],
                            scale=1.0)
                    # DVE pair ops + per-tile threshold, Pool pair reset
                    pb = slice(ph * B, (ph + gw) * B)
                    nc.vector.scalar_tensor_tensor(
                        u_l[:, pb], u_l[:, pb], 0.5, syn_t[:, :gw * B],
                        op0=OP.mult, op1=OP.add)
                    nc.vector.scalar_tensor_tensor(
                        v_l[:, pb], v_l[:, pb], 0.75, u_l[:, pb],
                        op0=OP.mult, op1=OP.add)
                    if gw == 2 and li != 2:
                        nc.vector.tensor_scalar(
                            hist[:, HOFF[li] + ph:HOFF[li] + ph + 2, t, :],
                            v_l[:, pb], 0.5, None, op0=OP.is_le)
                    else:
                        for si, tl in enumerate(grp):
                            M = tl['m1'] - tl['m0']
                            nc.vector.tensor_scalar(
                                hist[:M, HOFF[li] + ph + si, t, :],
                                v_l[:M, (ph + si) * B:(ph + si + 1) * B],
                                0.5, None, op0=OP.is_le)
                    nc.gpsimd.tensor_tensor(
                        v_l[:, pb], v_l[:, pb],
                        hist[:, HOFF[li] + ph, t, :] if gw == 1 else
                        hist[:, HOFF[li] + ph:HOFF[li] + ph + 2, t, :],
                        op=OP.mult)
                if li == 5:
                    if blk == 0 and t == 0:
                        psc = psum4.tile([128, 8], F32, tag="ps4")
                        for half in range(3):
                            nc.tensor.matmul(psc[:, 0:8], ones[0:1, :],
                                             c4s[0:1, half * 8:half * 8 + 8],
                                             start=(half == 0), stop=(half == 2))
                        nc.scalar.activation(c4sb[:], psc[:], AF.Copy)
                    ps4 = psum4.tile([128, 8], F32, tag="ps4")
                    h3 = hist[:, HOFF[5], t, :]
                    for j in range(4):
                        for half in range(3):
                            nc.tensor.matmul(ps4[:, j * 2:(j + 1) * 2],
                                             h3[:, j * 128:(j + 1) * 128],
                                             wf4[:, half * 2:half * 2 + 2],
                                             start=(half == 0), stop=(half == 2))
                    nc.vector.tensor_tensor(syn4[:], ps4[:], c4sb[:], op=OP.add)
                    nc.vector.scalar_tensor_tensor(u4[:], u4[:], 0.5, syn4[:],
                                                   op0=OP.mult, op1=OP.add)
                    nc.vector.scalar_tensor_tensor(v4[:], v4[:], 0.75, u4[:],
                                                   op0=OP.mult, op1=OP.add)
                    nc.vector.tensor_scalar(ns4[:], v4[:], 0.5, None,
                                            op0=OP.is_le)
                    nc.vector.tensor_tensor(v4[:], v4[:], ns4[:], op=OP.mult)
                    nc.vector.scalar_tensor_tensor(acc4[:], ns4[:], 1.0,
                                                   acc4[:], op0=OP.subtract,
                                                   op1=OP.add)

    out_sb = persist.tile([128, 8], F32, tag="outsb")
    nc.vector.tensor_scalar_mul(out_sb[:], acc4[:], -1.0 / T)
    nc.sync.dma_start(prm['out'][:], out_sb[:])


_CACHE = {}


def _get_nc(layers, wcols, ntiles):
    key = ('nc2', wcols, ntiles)
    if key in _CACHE:
        return _CACHE[key]
    nc = bacc_mod.Bacc()
    prm = dict(
        scan=nc.declare_dram_parameter("scan", [360, T * B], BF16,
                                       isOutput=False).rearrange(
                                           "l (t b) -> l t b", b=B),
        normal=nc.declare_dram_parameter("normal", [6, T * B], BF16,
                                         isOutput=False).rearrange(
                                             "l (t b) -> l t b", b=B),
        w=nc.declare_dram_parameter("w", [128, wcols], BF16, isOutput=False),
        bias=nc.declare_dram_parameter("bias", [128, ntiles], F32,
                                       isOutput=False),
        wf4=nc.declare_dram_parameter("wf4", [128, 6], BF16, isOutput=False),
        c4s=nc.declare_dram_parameter("c4s", [1, 24], BF16, isOutput=False),
        out=nc.declare_dram_parameter("out", [128, 8], F32, isOutput=True),
    )
    with tile.TileContext(nc) as tc:
        _emit(tc, layers, wcols, prm)
    nc.compile()
    _CACHE[key] = nc
    return nc


def build_nc_for_sim(inputs):
    inp = {k: np.asarray(v, np.float32) for k, v in inputs.items()
           if k.endswith('_w') or k.endswith('_b')}
    layers = _plan_layers(inp)
    w3, btab = _pack_weights(layers)
    return _get_nc(layers, w3.shape[1], btab.shape[1])


def kernel(normal_spikes, scan_spikes, batch_size,
           conv1_w, conv1_b, conv2_w, conv2_b, conv3_w, conv3_b,
           fc1_w, fc1_b, fc2_w, fc2_b, fc3_w, fc3_b, fc4_w, fc4_b):
    inp = dict(conv1_w=conv1_w, conv1_b=conv1_b, conv2_w=conv2_w,
               conv2_b=conv2_b, conv3_w=conv3_w, conv3_b=conv3_b,
               fc1_w=fc1_w, fc1_b=fc1_b, fc2_w=fc2_w, fc2_b=fc2_b,
               fc3_w=fc3_w, fc3_b=fc3_b, fc4_w=fc4_w, fc4_b=fc4_b)
    inp = {k: np.asarray(v, np.float32) for k, v in inp.items()}
    layers = _plan_layers(inp)
    w3, btab = _pack_weights(layers)
    wf4, c4s = _pack_fc4(inp)
    nc = _get_nc(layers, w3.shape[1], btab.shape[1])

    bf = ml_dtypes.bfloat16
    scan_t = np.ascontiguousarray(
        np.asarray(scan_spikes)[:, 0].transpose(1, 2, 0)).astype(bf)
    norm_t = np.ascontiguousarray(
        np.asarray(normal_spikes).transpose(1, 2, 0)).astype(bf)

    in_maps = []
    for c in range(N_CORES):
        sl = slice(c * B, (c + 1) * B)
        in_maps.append(dict(
            scan=np.ascontiguousarray(scan_t[:, :, sl]).reshape(360, T * B),
            normal=np.ascontiguousarray(norm_t[:, :, sl]).reshape(6, T * B),
            w=w3, bias=btab, wf4=wf4, c4s=c4s))
    import time as _time
    t0 = _time.time()
    res = run_bass_kernel_spmd(nc, in_maps, list(range(N_CORES)))
    wall1 = _time.time() - t0
    full = np.empty((B_FULL, 2), np.float32)
    for c in range(N_CORES):
        o = res.results[c]["out"]  # [128, 8]
        for j in range(4):
            full[c * B + j * 128:(c * B + (j + 1) * 128), :] = o[:, j * 2:(j + 1) * 2]
    kernel._last_exec_ns = res.exec_time_ns
    kernel._wall_exec_s = wall1
    return full
